# revision 1
# baseline (speedup 1.0000x reference)
"""ESPnet-style attention decoder (nn_Decoder) on 8 Trainium2 NeuronCores.

Strategy (8-way SPMD, one chip):
- Recurrence is 8-way tensor-parallel over the 4096 LSTM gate dim (512
  gates/core, grouped as 128 of each of i/f/g/o via a host-side row
  permutation), batch-parallel attention (4 sequences/core).
- Per decode step: AllGather(z0^T slice) and AllGather(att_c) chain the
  cores; LSTM1 trails one step off the critical path with its own
  AllGather(z1^T slice).
- The embedding x-contribution X0 = ey @ W_ih0[:, :1024]^T + biases is
  precomputed for all steps as one parallel matmul (bf16).
- Final phase: logits are output-dim-parallel (1250 vocab cols/core,
  f32r matmuls); per-row (local-max, local-sumexp, label-logit)
  partials are returned and the host merges them into loss/acc/ppl.

All PE operands are float32r (TF32-like, full rate) except the X0
precompute (bf16).  PSUM accumulation is fp32; LSTM cell state c stays
fp32 in SBUF and never leaves the core.
"""
import os
import sys

sys.path.insert(0, "/opt/trn_rl_repo")

import numpy as np
import ml_dtypes

import concourse.bass as bass
import concourse.tile as tile
from concourse import bacc, mybir
from concourse import bass_utils

f32 = mybir.dt.float32
f32r = mybir.dt.float32r
bf16 = mybir.dt.bfloat16
FT = mybir.ActivationFunctionType
OP = mybir.AluOpType
AX = mybir.AxisListType

NC = 8
B, T, EPROJS = 32, 512, 512
DUNITS, ODIM, ATT_DIM = 1024, 10000, 320
APAD = 384            # ATT_DIM padded to 3*128
L = 128
S = int(os.environ.get("DEC_STEPS", L + 1))   # decode steps (129)
SOS = EOS = ODIM - 1
BL = B // NC          # sequences per core (4)
GS = 4 * DUNITS // NC  # gate slice per core (512)
ZS = DUNITS // NC     # hidden slice per core (128)
OS = ODIM // NC       # vocab slice per core (1250)

_BUILD_CACHE = {}
_SKIP = set(os.environ.get("KSKIP", "").split(","))


def _sap(ap, start, step, count):
    """Partition-strided view: partitions start, start+step, ... of an AP."""
    a = ap[start: start + (count - 1) * step + 1]
    return bass.AP(tensor=a.tensor, offset=a.offset,
                   ap=[[step, count]] + [list(x) for x in a.ap[1:]])


def _cell(nc, W, g_sb, c_sb, tag):
    """LSTM cell on a [32, 512] gate slice (i|f|g|o blocks of 128).
    Updates c_sb in place, returns the new z slice [32, 128] (f32r)."""
    sif = W.tile([B, 256], f32, tag=tag + "sif")
    nc.scalar.activation(out=sif[:], in_=g_sb[:, 0:256], func=FT.Sigmoid)
    tg = W.tile([B, ZS], f32, tag=tag + "tg")
    nc.scalar.activation(out=tg[:], in_=g_sb[:, 256:384], func=FT.Tanh)
    so = W.tile([B, ZS], f32, tag=tag + "so")
    nc.scalar.activation(out=so[:], in_=g_sb[:, 384:512], func=FT.Sigmoid)
    t1 = W.tile([B, ZS], f32, tag=tag + "t1")
    nc.vector.tensor_mul(out=t1[:], in0=sif[:, 128:256], in1=c_sb[:])
    t2 = W.tile([B, ZS], f32, tag=tag + "t2")
    nc.vector.tensor_mul(out=t2[:], in0=sif[:, 0:128], in1=tg[:])
    nc.vector.tensor_add(out=c_sb[:], in0=t1[:], in1=t2[:])
    tc_ = W.tile([B, ZS], f32, tag=tag + "tc")
    nc.scalar.activation(out=tc_[:], in_=c_sb[:], func=FT.Tanh)
    zn = W.tile([B, ZS], f32r, tag=tag + "zn")
    nc.vector.tensor_mul(out=zn[:], in0=so[:], in1=tc_[:])
    return zn


def build(steps):
    nrow = steps * B
    nch = (nrow + 127) // 128
    tpad = 4 * nch

    nc = bacc.Bacc("TRN2", target_bir_lowering=False, debug=False,
                   num_devices=NC)

    def din(name, shape, dt):
        return nc.dram_tensor(name, shape, dt, kind="ExternalInput")

    hs_nat = din("hs_nat", (128, BL, 4, EPROJS), f32r)
    hsT = din("hsT", (128, 4, BL * T), f32r)
    eysT = din("eysT", (128, 8, nrow), bf16)
    wih0pT = din("wih0pT", (128, 8, GS), bf16)
    x0bias = din("x0bias", (1, GS), f32)
    wencT = din("wencT", (128, 4, APAD), f32r)
    bencp = din("bencp", (128, 3), f32)
    wdecT = din("wdecT", (128, 8, APAD), f32r)
    wattT = din("wattT", (128, 4, GS), f32r)
    whh0T = din("whh0T", (128, 8, GS), f32r)
    wih1T = din("wih1T", (128, 8, GS), f32r)
    whh1T = din("whh1T", (128, 8, GS), f32r)
    bias1 = din("bias1", (1, GS), f32)
    maskb = din("maskb", (BL, T), f32)
    sel = din("sel", (B, BL), f32r)
    woutT = din("woutT", (128, 8, OS), f32r)
    boutsl = din("boutsl", (1, OS), f32)
    labels = din("labels", (128, nch), f32)
    ident = din("ident", (128, 128), f32r)
    zinit = din("zinit", (128, 8, B), f32r)

    out_stats = nc.dram_tensor("out_stats", (128, nch, 3), f32,
                               kind="ExternalOutput")

    rg = [list(range(NC))]

    with tile.TileContext(nc) as tc:
        with tc.tile_pool(name="dram", bufs=1, space="DRAM") as DR:
            zs_dram = DR.tile([tpad, 128, 8, B], f32r, tag="zs")
            x0_dram = DR.tile([steps, B, GS], f32, tag="x0")

            with tc.tile_pool(name="persist", bufs=1) as P:
                # ------------- persistent SBUF -------------
                hs_sb = P.tile([128, BL, 4, EPROJS], f32r)
                nc.sync.dma_start(hs_sb[:], hs_nat[:])
                wdecT_sb = P.tile([128, 8, APAD], f32r)
                nc.sync.dma_start(wdecT_sb[:], wdecT[:])
                wattT_sb = P.tile([128, 4, GS], f32r)
                nc.sync.dma_start(wattT_sb[:], wattT[:])
                whh0T_sb = P.tile([128, 8, GS], f32r)
                nc.sync.dma_start(whh0T_sb[:], whh0T[:])
                wih1T_sb = P.tile([128, 8, GS], f32r)
                nc.sync.dma_start(wih1T_sb[:], wih1T[:])
                whh1T_sb = P.tile([128, 8, GS], f32r)
                nc.sync.dma_start(whh1T_sb[:], whh1T[:])
                bias1_sb = P.tile([B, GS], f32)
                nc.sync.dma_start(
                    bias1_sb[:],
                    bass.AP(tensor=bias1.ap().tensor, offset=0,
                            ap=[[0, B], [1, GS]]))
                maskb_sb = P.tile([BL, T], f32)
                nc.sync.dma_start(maskb_sb[:], maskb[:])
                sel_sb = P.tile([B, BL], f32r)
                nc.sync.dma_start(sel_sb[:], sel[:])
                ident_sb = P.tile([128, 128], f32r)
                nc.sync.dma_start(ident_sb[:], ident[:])
                pre_encT_sb = P.tile([128, 3, BL * T], f32r)

                z0T_sb = P.tile([128, 8, B], f32r)
                nc.sync.dma_start(z0T_sb[:], zinit[:])
                z1T_sb = P.tile([128, 8, B], f32r)
                nc.sync.dma_start(z1T_sb[:], zinit[:])
                c0_sb = P.tile([B, ZS], f32)
                nc.vector.memset(c0_sb[:], 0.0)
                c1_sb = P.tile([B, ZS], f32)
                nc.vector.memset(c1_sb[:], 0.0)

                # ------------- prologue A: pre_enc -------------
                if "pre" not in _SKIP:
                 with (
                    tc.tile_pool(name="prA", bufs=1) as PA,
                    tc.tile_pool(name="prAps", bufs=1, space="PSUM") as PAP,
                ):
                    hsT_sb = PA.tile([128, 4, BL * T], f32r, tag="hsT")
                    nc.sync.dma_start(hsT_sb[:], hsT[:])
                    wencT_sb = PA.tile([128, 4, APAD], f32r, tag="wenc")
                    nc.sync.dma_start(wencT_sb[:], wencT[:])
                    bencp_sb = PA.tile([128, 3], f32, tag="benc")
                    nc.sync.dma_start(bencp_sb[:], bencp[:])
                    for ac in range(3):
                        ps = PAP.tile([128, BL * T], f32, tag="pe")
                        for dk in range(4):
                            for ns in range(4):
                                nc.tensor.matmul(
                                    ps[:, ns * 512:(ns + 1) * 512],
                                    wencT_sb[:, dk, ac * 128:(ac + 1) * 128],
                                    hsT_sb[:, dk, ns * 512:(ns + 1) * 512],
                                    start=(dk == 0), stop=(dk == 3))
                        nc.scalar.activation(
                            out=pre_encT_sb[:, ac, :], in_=ps[:],
                            func=FT.Tanh, bias=bencp_sb[:, ac:ac + 1],
                            scale=1.0)

                # ------------- prologue B: X0 precompute -------------
                if "x0" not in _SKIP:
                 with (
                    tc.tile_pool(name="prB", bufs=2) as PB,
                    tc.tile_pool(name="prB1", bufs=1) as PB1,
                    tc.tile_pool(name="prBps", bufs=2, space="PSUM") as PBP,
                ):
                    wih0pT_sb = PB1.tile([128, 8, GS], bf16, tag="wih0p")
                    nc.sync.dma_start(wih0pT_sb[:], wih0pT[:])
                    x0bias_sb = PB1.tile([128, GS], f32, tag="x0b")
                    nc.sync.dma_start(
                        x0bias_sb[:],
                        bass.AP(tensor=x0bias.ap().tensor, offset=0,
                                ap=[[0, 128], [1, GS]]))
                    x0_flat = x0_dram[:].rearrange("t b g -> (t b) g")
                    for ch in range(nch):
                        cw = min(128, nrow - ch * 128)
                        ey_t = PB.tile([128, 8, 128], bf16, tag="eych")
                        nc.sync.dma_start(
                            ey_t[:, :, :cw],
                            eysT[:, :, ch * 128: ch * 128 + cw])
                        ps = PBP.tile([128, GS], f32, tag="x0")
                        for kt in range(8):
                            nc.tensor.matmul(
                                ps[:cw, :], ey_t[:, kt, :cw],
                                wih0pT_sb[:, kt, :],
                                start=(kt == 0), stop=(kt == 7))
                        g = PB.tile([128, GS], f32, tag="x0g")
                        nc.vector.tensor_tensor(
                            out=g[:cw, :], in0=ps[:cw, :],
                            in1=x0bias_sb[:cw, :], op=OP.add)
                        nc.sync.dma_start(
                            x0_flat[ch * 128: ch * 128 + cw, :], g[:cw, :])
                    # zero the zs padding slots
                    for tp in range(steps, tpad):
                        nc.sync.dma_start(zs_dram[tp], zinit[:])

                # ------------- recurrence -------------
                if "rec" not in _SKIP:
                 with (
                    tc.tile_pool(name="work", bufs=2) as W,
                    tc.tile_pool(name="ps_big", bufs=1, space="PSUM") as PSbig,
                    tc.tile_pool(name="ps_sm", bufs=1, space="PSUM") as PSsm,
                    tc.tile_pool(name="ps_g", bufs=1, space="PSUM") as PSg,
                    tc.tile_pool(name="ps_zT", bufs=1, space="PSUM") as PSzT,
                    tc.tile_pool(name="bnc", bufs=2, space="DRAM") as BN,
                    tc.tile_pool(name="shr", bufs=2, space="DRAM") as SH,
                    tc.tile_pool(name="x0pre", bufs=2) as X0P,
                    tc.tile_pool(name="wbig", bufs=1) as WB,
                ):
                    for t in range(steps):
                        x0_t = X0P.tile([B, GS], f32, tag="x0t")
                        nc.sync.dma_start(x0_t[:], x0_dram[t])

                        skip_attn = "attn" in _SKIP
                        if not skip_attn:
                            # dec = tanh(z0 @ WdecT), all 32 seqs
                            dec_ps = PSsm.tile([B, APAD], f32, tag="sm")
                            for kt in range(8):
                                nc.tensor.matmul(dec_ps[:], z0T_sb[:, kt, :],
                                                 wdecT_sb[:, kt, :],
                                                 start=(kt == 0), stop=(kt == 7))
                            dec_sb = W.tile([B, APAD], f32r, tag="dec")
                            nc.scalar.activation(out=dec_sb[:], in_=dec_ps[:],
                                                 func=FT.Tanh)

                            # decT compact [128, 3, 4] via selector matmul
                            dT_ps = PSsm.tile([128, 3, BL], f32, tag="sm")
                            for ac in range(3):
                                nc.tensor.matmul(
                                    dT_ps[:, ac, :],
                                    dec_sb[:, ac * 128:(ac + 1) * 128],
                                    sel_sb[:], start=True, stop=True)
                            decT_sb = W.tile([128, 3, BL], f32r, tag="dT")
                            nc.vector.tensor_copy(out=decT_sb[:], in_=dT_ps[:])

                            # e diag-packed [4, (s,t)]
                            e_ps = PSbig.tile([BL, BL * T], f32, tag="big")
                            for sj in range(BL):
                                for ac in range(3):
                                    nc.tensor.matmul(
                                        e_ps[:, sj * T:(sj + 1) * T],
                                        decT_sb[:, ac, :],
                                        pre_encT_sb[:, ac, sj * T:(sj + 1) * T],
                                        start=(ac == 0), stop=(ac == 2))
                            e_sb = WB.tile([BL, BL * T], f32, tag="esb")
                            nc.vector.tensor_copy(out=e_sb[:], in_=e_ps[:])
                            e_m = W.tile([BL, T], f32, tag="em")
                            for j in range(BL):
                                nc.sync.dma_start(
                                    e_m[j:j + 1, :],
                                    e_sb[j:j + 1, j * T:(j + 1) * T])

                            # softmax over T: w = softmax(2*e + maskb)
                            e_b = W.tile([BL, T], f32, tag="eb")
                            nc.vector.scalar_tensor_tensor(
                                out=e_b[:], in0=e_m[:], scalar=2.0,
                                in1=maskb_sb[:], op0=OP.mult, op1=OP.add)
                            negm = W.tile([BL, 1], f32, tag="negm")
                            nc.vector.tensor_reduce(
                                out=negm[:], in_=e_b[:], op=OP.max, axis=AX.X,
                                negate=True)
                            p_t = W.tile([BL, T], f32, tag="pt")
                            ssum = W.tile([BL, 1], f32, tag="ssum")
                            nc.scalar.activation(
                                out=p_t[:], in_=e_b[:], func=FT.Exp,
                                bias=negm[:], scale=1.0, accum_out=ssum[:])
                            rsum = W.tile([BL, 1], f32, tag="rsum")
                            nc.vector.reciprocal(out=rsum[:], in_=ssum[:])
                            w_t = W.tile([BL, T], f32r, tag="wt")
                            nc.vector.tensor_scalar_mul(out=w_t[:], in0=p_t[:],
                                                        scalar1=rsum[:])

                            # wT (4 transposes [4,128] -> [128,4])
                            wT_ps = PSsm.tile([128, 4, BL], f32r, tag="sm")
                            for tk in range(4):
                                nc.tensor.transpose(
                                    wT_ps[:, tk, :],
                                    w_t[:, tk * 128:(tk + 1) * 128],
                                    ident_sb[0:BL, 0:BL])
                            wT_sb = W.tile([128, 4, BL], f32r, tag="wT")
                            nc.vector.tensor_copy(out=wT_sb[:], in_=wT_ps[:])

                            # att_c diag-packed [4, (s,d)]
                            ac_ps = PSbig.tile([BL, BL * EPROJS], f32, tag="big")
                            for sj in range(BL):
                                for tk in range(4):
                                    nc.tensor.matmul(
                                        ac_ps[:, sj * EPROJS:(sj + 1) * EPROJS],
                                        wT_sb[:, tk, :],
                                        hs_sb[:, sj, tk, :],
                                        start=(tk == 0), stop=(tk == 3))
                            ac_sb = WB.tile([BL, BL * EPROJS], f32, tag="acsb")
                            nc.vector.tensor_copy(out=ac_sb[:], in_=ac_ps[:])

                            # AllGather att_c -> [32, 512] (diag rows to bounce)
                            acb_in = BN.tile([BL, EPROJS], f32, tag="acb")
                            for j in range(BL):
                                nc.sync.dma_start(
                                    acb_in[j:j + 1, :],
                                    ac_sb[j:j + 1, j * EPROJS:(j + 1) * EPROJS])
                            acb_out = SH.tile([B, EPROJS], f32, tag="acs",
                                              addr_space="Shared")
                            nc.gpsimd.collective_compute(
                                "AllGather", OP.bypass, replica_groups=rg,
                                ins=[acb_in[:]], outs=[acb_out[:]])
                            attall_sb = W.tile([B, EPROJS], f32, tag="attall")
                            nc.sync.dma_start(attall_sb[:], acb_out[:])

                            # attT (4 transposes [32,128] -> [128,32])
                            aT_ps = PSsm.tile([128, 4, B], f32, tag="sm2")
                            for dk in range(4):
                                nc.tensor.transpose(
                                    aT_ps[:, dk, :],
                                    attall_sb[:, dk * 128:(dk + 1) * 128],
                                    ident_sb[0:B, 0:B].bitcast(f32))
                            attT_sb = W.tile([128, 4, B], f32r, tag="attT")
                            nc.vector.tensor_copy(out=attT_sb[:], in_=aT_ps[:])

                        # g0 = att_c @ WattT + z0 @ Whh0T  (+ X0[t])
                        g0_ps = PSg.tile([B, GS], f32, tag="g")
                        if not skip_attn:
                            for dk in range(4):
                                nc.tensor.matmul(g0_ps[:], attT_sb[:, dk, :],
                                                 wattT_sb[:, dk, :],
                                                 start=(dk == 0), stop=False)
                        for kt in range(8):
                            nc.tensor.matmul(g0_ps[:], z0T_sb[:, kt, :],
                                             whh0T_sb[:, kt, :],
                                             start=(skip_attn and kt == 0),
                                             stop=(kt == 7))
                        g0_sb = W.tile([B, GS], f32, tag="g0")
                        nc.vector.tensor_tensor(out=g0_sb[:], in0=g0_ps[:],
                                                in1=x0_t[:], op=OP.add)

                        z0n = _cell(nc, W, g0_sb, c0_sb, "cl")

                        # z0 slice -> [128, 32] -> AllGather -> z0T full
                        z0T_ps = PSzT.tile([128, B], f32r, tag="zT")
                        nc.tensor.transpose(z0T_ps[:], z0n[:],
                                            ident_sb[0:B, 0:B])
                        z0Tsl = W.tile([128, B], f32, tag="z0Tsl")
                        nc.vector.tensor_copy(out=z0Tsl[:], in_=z0T_ps[:])
                        z0b_in = BN.tile([128, B], f32, tag="z0b")
                        nc.sync.dma_start(z0b_in[:], z0Tsl[:])
                        z0b_out = SH.tile([128 * NC, B], f32, tag="z0s",
                                          addr_space="Shared")
                        nc.gpsimd.collective_compute(
                            "AllGather", OP.bypass, replica_groups=rg,
                            ins=[z0b_in[:]], outs=[z0b_out[:]])
                        z0g_sb = W.tile([128, 8, B], f32, tag="z0g")
                        nc.sync.dma_start(
                            z0g_sb[:],
                            z0b_out[:].rearrange("(kt k) b -> k kt b", k=128))
                        nc.vector.tensor_copy(out=z0T_sb[:], in_=z0g_sb[:])

                        # LSTM1 (fresh z0T, previous z1T)
                        if "lstm1" in _SKIP:
                            nc.sync.dma_start(zs_dram[t], z1T_sb[:])
                            continue
                        g1_ps = PSg.tile([B, GS], f32, tag="g")
                        for kt in range(8):
                            nc.tensor.matmul(g1_ps[:], z0T_sb[:, kt, :],
                                             wih1T_sb[:, kt, :],
                                             start=(kt == 0), stop=False)
                        for kt in range(8):
                            nc.tensor.matmul(g1_ps[:], z1T_sb[:, kt, :],
                                             whh1T_sb[:, kt, :],
                                             start=False, stop=(kt == 7))
                        g1_sb = W.tile([B, GS], f32, tag="g1")
                        nc.vector.tensor_tensor(
                            out=g1_sb[:], in0=g1_ps[:],
                            in1=bias1_sb[:], op=OP.add)
                        z1n = _cell(nc, W, g1_sb, c1_sb, "cl")

                        z1T_ps = PSzT.tile([128, B], f32r, tag="zT")
                        nc.tensor.transpose(z1T_ps[:], z1n[:],
                                            ident_sb[0:B, 0:B])
                        z1Tsl = W.tile([128, B], f32, tag="z1Tsl")
                        nc.vector.tensor_copy(out=z1Tsl[:], in_=z1T_ps[:])
                        z1b_in = BN.tile([128, B], f32, tag="z1b")
                        nc.sync.dma_start(z1b_in[:], z1Tsl[:])
                        z1b_out = SH.tile([128 * NC, B], f32, tag="z1s",
                                          addr_space="Shared")
                        nc.gpsimd.collective_compute(
                            "AllGather", OP.bypass, replica_groups=rg,
                            ins=[z1b_in[:]], outs=[z1b_out[:]])
                        z1g_sb = W.tile([128, 8, B], f32, tag="z1g")
                        nc.sync.dma_start(
                            z1g_sb[:],
                            z1b_out[:].rearrange("(kt k) b -> k kt b", k=128))
                        nc.vector.tensor_copy(out=z1T_sb[:], in_=z1g_sb[:])
                        nc.sync.dma_start(zs_dram[t], z1T_sb[:])

            # ------------- logits + partial log-softmax -------------
            if "log" not in _SKIP:
             with (
                tc.tile_pool(name="lg", bufs=2) as LG,
                tc.tile_pool(name="lg1", bufs=1) as LG1,
                tc.tile_pool(name="lgps", bufs=2, space="PSUM") as LPS,
            ):
                woutT_sb = LG1.tile([128, 8, OS], f32r, tag="wout")
                nc.sync.dma_start(woutT_sb[:], woutT[:])
                bout_sb = LG1.tile([128, OS], f32, tag="bout")
                nc.sync.dma_start(
                    bout_sb[:],
                    bass.AP(tensor=boutsl.ap().tensor, offset=0,
                            ap=[[0, 128], [1, OS]]))
                lab_sb = LG1.tile([128, nch], f32, tag="lab")
                nc.sync.dma_start(lab_sb[:], labels[:])
                iota_sb = LG1.tile([128, OS], f32, tag="iota")
                nc.gpsimd.iota(iota_sb[:], pattern=[[1, OS]], base=0,
                               channel_multiplier=0,
                               allow_small_or_imprecise_dtypes=True)
                m_all = LG1.tile([128, nch], f32, tag="m")
                s_all = LG1.tile([128, nch], f32, tag="s")
                lg_all = LG1.tile([128, nch], f32, tag="lg")

                osubs = [(0, 512), (512, 512), (1024, OS - 1024)]
                for ch in range(nch):
                    zch = LG.tile([128, 8, 4, B], f32r, tag="zch")
                    nc.sync.dma_start(
                        zch[:],
                        zs_dram[4 * ch: 4 * ch + 4]
                        .rearrange("t k kt b -> k kt t b"))
                    zch_f = zch[:].rearrange("k kt t b -> k kt (t b)")
                    ps = LPS.tile([128, OS], f32, tag="lps")
                    for (o0, ow) in osubs:
                        for kt in range(8):
                            nc.tensor.matmul(
                                ps[:, o0:o0 + ow], zch_f[:, kt, :],
                                woutT_sb[:, kt, o0:o0 + ow],
                                start=(kt == 0), stop=(kt == 7))
                    buf = LG.tile([128, OS], f32, tag="lbuf")
                    nc.vector.tensor_tensor(
                        out=buf[:], in0=ps[:],
                        in1=bout_sb[:], op=OP.add)
                    negm = LG.tile([128, 1], f32, tag="lnegm")
                    nc.vector.tensor_reduce(out=negm[:], in_=buf[:],
                                            op=OP.max, axis=AX.X, negate=True)
                    nc.vector.tensor_scalar_mul(
                        out=m_all[:, ch:ch + 1], in0=negm[:], scalar1=-1.0)
                    if "lmask" not in _SKIP:
                        mask = LG.tile([128, OS], f32, tag="lmask")
                        nc.vector.tensor_scalar(
                            out=mask[:], in0=iota_sb[:],
                            scalar1=lab_sb[:, ch:ch + 1], scalar2=None,
                            op0=OP.is_equal)
                        prod = LG.tile([128, OS], f32, tag="lprod")
                        nc.vector.tensor_mul(out=prod[:], in0=buf[:],
                                             in1=mask[:])
                        nc.vector.tensor_reduce(
                            out=lg_all[:, ch:ch + 1], in_=prod[:],
                            op=OP.add, axis=AX.X)
                    if "lexp" not in _SKIP:
                        nc.scalar.activation(
                            out=buf[:], in_=buf[:], func=FT.Exp,
                            bias=negm[:], scale=1.0,
                            accum_out=s_all[:, ch:ch + 1])

                nc.sync.dma_start(out_stats[:, :, 0], m_all[:])
                nc.sync.dma_start(
                    out_stats[:, :, 1],
                    m_all[:] if "lexp" in _SKIP else s_all[:])
                nc.sync.dma_start(
                    out_stats[:, :, 2],
                    m_all[:] if "lmask" in _SKIP else lg_all[:])

    nc.finalize()
    return nc


# ---------------------------------------------------------------------------
# host side
# ---------------------------------------------------------------------------

def _prep_inputs(hs_pad, hlens, ys_pad, embed_w, Wenc, benc, Wdec,
                 W_ih0, W_hh0, b_ih0, b_hh0, W_ih1, W_hh1, b_ih1, b_hh1,
                 Wout, bout, steps):
    """Shard + pack all inputs into per-core in_maps (pure data movement)."""
    f = np.float32
    hs_pad = np.asarray(hs_pad, f)
    ys_pad = np.asarray(ys_pad)
    ys_in = np.concatenate(
        [np.full((B, 1), SOS, ys_pad.dtype), ys_pad], axis=1)[:, :steps]
    ys_out = np.concatenate(
        [ys_pad, np.full((B, 1), EOS, ys_pad.dtype)], axis=1)[:, :steps]

    # gate permutation: core c's rows = 128 each of i/f/g/o
    perm = np.concatenate(
        [g * DUNITS + c * ZS + np.arange(ZS)
         for c in range(NC) for g in range(4)])

    eys = np.asarray(embed_w, f)[ys_in]                  # [B, steps, 1024]
    eysT = np.ascontiguousarray(
        eys.transpose(2, 1, 0).reshape(DUNITS, steps * B))
    eysT = np.ascontiguousarray(
        eysT.reshape(8, 128, -1).transpose(1, 0, 2)).astype(
            ml_dtypes.bfloat16)                          # [128, 8, rows]

    def kpack(M, dt=f):
        """[K, N] -> [128, K//128, N]"""
        K = M.shape[0]
        return np.ascontiguousarray(
            M.reshape(K // 128, 128, -1).transpose(1, 0, 2)).astype(dt)

    W_ih0 = np.asarray(W_ih0, f)[perm]
    W_hh0 = np.asarray(W_hh0, f)[perm]
    W_ih1 = np.asarray(W_ih1, f)[perm]
    W_hh1 = np.asarray(W_hh1, f)[perm]
    bias0 = (np.asarray(b_ih0, f) + np.asarray(b_hh0, f))[perm]
    bias1v = (np.asarray(b_ih1, f) + np.asarray(b_hh1, f))[perm]

    wencp = np.zeros((APAD, EPROJS), f)
    wencp[:ATT_DIM] = np.asarray(Wenc, f)
    bencpv = np.zeros((3, 128), f)
    bencpv.reshape(-1)[:ATT_DIM] = np.asarray(benc, f)
    wdecp = np.zeros((APAD, DUNITS), f)
    wdecp[:ATT_DIM] = np.asarray(Wdec, f)

    wencT = kpack(wencp.T)                      # [128, 4, 384]
    wdecT = kpack(wdecp.T)                      # [128, 8, 384]
    identv = np.eye(128, dtype=f)
    zinitv = np.zeros((128, 8, B), f)

    Wout = np.asarray(Wout, f)
    bout_v = np.asarray(bout, f)

    ys_out_flat = ys_out.T.reshape(-1)          # row r = t*B + b
    nrow = steps * B
    nch = (nrow + 127) // 128

    in_maps = []
    for c in range(NC):
        sl = slice(GS * c, GS * (c + 1))
        seqs = slice(BL * c, BL * (c + 1))
        hs_c = hs_pad[seqs]                     # [4, 512, 512]
        hs_nat = np.ascontiguousarray(
            hs_c.reshape(BL, 4, 128, EPROJS).transpose(2, 0, 1, 3))
        hsT = np.ascontiguousarray(
            hs_c.transpose(2, 0, 1)             # [d, s, t]
            .reshape(4, 128, BL, T)
            .transpose(1, 0, 2, 3)
            .reshape(128, 4, BL * T))
        hl = np.asarray(hlens).reshape(-1)[seqs]
        maskbv = np.where(np.arange(T)[None, :] < hl[:, None],
                          0.0, -1e10).astype(f)
        selv = np.zeros((B, BL), f)
        for j in range(BL):
            selv[BL * c + j, j] = 1.0
        labv = np.full((nch * 128,), -1.0, f)
        lo = OS * c
        lb = ys_out_flat.astype(np.int64) - lo
        valid = (lb >= 0) & (lb < OS)
        labv[:nrow][valid] = lb[valid].astype(f)
        labv = labv.reshape(nch, 128).T.copy()  # [128, nch]

        in_maps.append({
            "hs_nat": hs_nat,
            "hsT": hsT,
            "eysT": eysT,
            "wih0pT": kpack(W_ih0[sl, :DUNITS].T, ml_dtypes.bfloat16),
            "x0bias": np.ascontiguousarray(bias0[sl][None]),
            "wencT": wencT,
            "bencp": np.ascontiguousarray(bencpv.T),
            "wdecT": wdecT,
            "wattT": kpack(W_ih0[sl, DUNITS:].T),
            "whh0T": kpack(W_hh0[sl].T),
            "wih1T": kpack(W_ih1[sl].T),
            "whh1T": kpack(W_hh1[sl].T),
            "bias1": np.ascontiguousarray(bias1v[sl][None]),
            "maskb": maskbv,
            "sel": selv,
            "woutT": kpack(Wout[OS * c: OS * (c + 1)].T),
            "boutsl": np.ascontiguousarray(bout_v[OS * c: OS * (c + 1)][None]),
            "labels": labv,
            "ident": identv,
            "zinit": zinitv,
        })
    return in_maps


def _combine(results, steps):
    """Merge per-core (m, S, lab) partials into (loss, acc, ppl)."""
    nrow = steps * B
    ms, ss, labs = [], [], []
    for r in results:
        st = r["out_stats"]                     # [128, nch, 3]
        ms.append(st[:, :, 0].T.reshape(-1)[:nrow])
        ss.append(st[:, :, 1].T.reshape(-1)[:nrow])
        labs.append(st[:, :, 2].T.reshape(-1)[:nrow])
    m = np.stack(ms)
    s = np.stack(ss)
    lab = np.stack(labs)
    gmax = m.max(axis=0)
    gsum = (s.astype(np.float64)
            * np.exp(m.astype(np.float64) - gmax[None])).sum(axis=0)
    lablogit = lab.sum(axis=0)
    nll = gmax.astype(np.float64) + np.log(gsum) - lablogit
    match = (lab == gmax[None]).any(axis=0)
    loss = np.float32(nll.mean() * L)
    acc = np.float32(match.mean())
    ppl = np.float32(np.exp(np.float64(loss) / B))
    return loss, acc, ppl


def kernel(**inputs):
    steps = S
    in_maps = _prep_inputs(steps=steps, **inputs)
    if steps not in _BUILD_CACHE:
        _BUILD_CACHE[steps] = build(steps)
    nc = _BUILD_CACHE[steps]
    res = bass_utils.run_bass_kernel_spmd(
        nc, in_maps, core_ids=list(range(NC)))
    return _combine(res.results, steps)



# revision 30
# speedup vs baseline: 1.4896x; 1.4896x over previous
"""ESPnet-style attention decoder (nn_Decoder) on 8 Trainium2 NeuronCores.

Strategy (8-way SPMD, one chip):
- Recurrence is 8-way tensor-parallel over the 4096 LSTM gate dim (512
  gates/core, grouped as 128 of each of i/f/g/o via a host-side row
  permutation), batch-parallel attention (4 sequences/core).
- Two AllGathers per decode step: (1) att context, pre-transposed so the
  post-collective path is DMA -> matmul; (2) a merged gather carrying
  [z0(t) | z1(t-1)] hidden-slice columns.
- Sigmoid is eliminated via sigma(x) = (1+tanh(x/2))/2 with the 0.5
  factors folded into the weights host-side (states stored as 2c, 2z),
  so ACT only ever needs the exp/tanh table set (no table reloads).
- e and att_c are computed with M=1-row matmuls writing each sequence's
  row directly (no diagonal extraction DMAs).
- PE-path operands are bf16 (same PE rate as f32r, fp32 accumulation);
  softmax, cell states and log-softmax stats stay fp32.
- The output-projection/log-softmax phase streams inside the recurrence
  (one 128-row chunk every 4th step) reading z1 from SBUF slots,
  filling collective shadows; per-row (max, sumexp, label-logit)
  partials are returned and merged host-side into loss/acc/ppl.
"""
import os
import sys

sys.path.insert(0, "/opt/trn_rl_repo")

import numpy as np
import ml_dtypes

import concourse.bass as bass
import concourse.tile as tile
from concourse import bacc, mybir
from concourse import bass_utils

f32 = mybir.dt.float32
f32r = mybir.dt.float32r
bf16 = mybir.dt.bfloat16
FT = mybir.ActivationFunctionType
OP = mybir.AluOpType
AX = mybir.AxisListType

NC = 8
B, T, EPROJS = 32, 512, 512
DUNITS, ODIM, ATT_DIM = 1024, 10000, 320
APAD = 384            # ATT_DIM padded to 3*128
L = 128
S = int(os.environ.get("DEC_STEPS", L + 1))   # decode steps (129)
SOS = EOS = ODIM - 1
BL = B // NC          # sequences per core (4)
GS = 4 * DUNITS // NC  # gate slice per core (512)
ZS = DUNITS // NC     # hidden slice per core (128)
OS = ODIM // NC       # vocab slice per core (1250)

_BUILD_CACHE = {}
_SKIP = set(os.environ.get("KSKIP", "").split(","))
WARM = int(os.environ.get("DEC_WARM", 0))


def _sap(ap, start, step, count):
    """Partition-strided view: partitions start, start+step, ... of an AP."""
    a = ap[start: start + (count - 1) * step + 1]
    return bass.AP(tensor=a.tensor, offset=a.offset,
                   ap=[[step, count]] + [list(x) for x in a.ap[1:]])


def _cell(nc, W, g_sb, c_sb, tag):
    """LSTM cell on a [32, 512] gate slice (i|f|g|o blocks of 128).

    Uses sigma(x) = (1+tanh(x/2))/2 with states stored doubled:
    c_sb holds C = 2c, returns Z = 2z (weights consuming z are halved
    host-side).  Updates c_sb in place; returns new Z slice [32,128] f32r.
    """
    tif = W.tile([B, 256], f32, tag=tag + "tif")
    nc.scalar.activation(out=tif[:], in_=g_sb[:, 0:256], func=FT.Tanh,
                         scale=0.5)
    tg = W.tile([B, ZS], f32, tag=tag + "tg")
    nc.scalar.activation(out=tg[:], in_=g_sb[:, 256:384], func=FT.Tanh)
    to = W.tile([B, ZS], f32, tag=tag + "to")
    nc.scalar.activation(out=to[:], in_=g_sb[:, 384:512], func=FT.Tanh,
                         scale=0.5)
    # C' = 0.5*(1+tf)*C + (1+ti)*tg
    u = W.tile([B, ZS], f32, tag=tag + "u")
    nc.vector.scalar_tensor_tensor(
        out=u[:], in0=tif[:, 128:256], scalar=1.0, in1=c_sb[:],
        op0=OP.add, op1=OP.mult)
    v = W.tile([B, ZS], f32, tag=tag + "v")
    nc.vector.scalar_tensor_tensor(
        out=v[:], in0=tif[:, 0:128], scalar=1.0, in1=tg[:],
        op0=OP.add, op1=OP.mult)
    nc.vector.scalar_tensor_tensor(
        out=c_sb[:], in0=u[:], scalar=0.5, in1=v[:],
        op0=OP.mult, op1=OP.add)
    tc_ = W.tile([B, ZS], f32, tag=tag + "tc")
    nc.scalar.activation(out=tc_[:], in_=c_sb[:], func=FT.Tanh, scale=0.5)
    zn = W.tile([B, ZS], f32r, tag=tag + "zn")
    nc.vector.scalar_tensor_tensor(
        out=zn[:], in0=to[:], scalar=1.0, in1=tc_[:],
        op0=OP.add, op1=OP.mult)
    return zn


def build(steps):
    nrow = steps * B
    nch = (nrow + 127) // 128

    nc = bacc.Bacc("TRN2", target_bir_lowering=False, debug=False,
                   num_devices=NC)

    def din(name, shape, dt):
        return nc.dram_tensor(name, shape, dt, kind="ExternalInput")

    hs_nat = din("hs_nat", (128, BL, 4, EPROJS), bf16)
    hsT = din("hsT", (128, 4, BL * T), bf16)
    eysT = din("eysT", (128, 8, nrow), bf16)
    wih0pT = din("wih0pT", (128, 8, GS), bf16)
    x0bias = din("x0bias", (1, GS), f32)
    wencT = din("wencT", (128, 4, APAD), bf16)
    bencp = din("bencp", (128, 3), f32)
    wdecT = din("wdecT", (128, 8, APAD), bf16)
    wattT = din("wattT", (128, 4, GS), bf16)
    whh0T = din("whh0T", (128, 8, GS), bf16)
    wih1T = din("wih1T", (128, 8, GS), bf16)
    whh1T = din("whh1T", (128, 8, GS), bf16)
    bias1 = din("bias1", (1, GS), bf16)
    maskb = din("maskb", (BL, T), f32)
    sel = din("sel", (B, BL), bf16)
    woutT = din("woutT", (128, 8, OS), bf16)
    boutsl = din("boutsl", (1, OS), f32)
    labels = din("labels", (128, nch), f32)
    ident = din("ident", (128, 128), f32r)
    zinit = din("zinit", (128, 8, 2 * B), bf16)

    out_stats = nc.dram_tensor("out_stats", (128, nch, 3), f32,
                               kind="ExternalOutput")

    rg = [list(range(NC))]

    with tile.TileContext(nc) as tc:
        with tc.tile_pool(name="dram", bufs=1, space="DRAM") as DR:
            x0_dram = DR.tile([steps, B, GS], bf16, tag="x0")

            with tc.tile_pool(name="persist", bufs=1) as P:
                # ------------- persistent SBUF -------------
                hs_sb = P.tile([128, BL, 4, EPROJS], bf16)
                nc.sync.dma_start(hs_sb[:], hs_nat[:])
                wdecT_sb = P.tile([128, 8, APAD], bf16)
                nc.sync.dma_start(wdecT_sb[:], wdecT[:])
                wattT_sb = P.tile([128, 4, GS], bf16)
                nc.sync.dma_start(wattT_sb[:], wattT[:])
                whh0T_sb = P.tile([128, 8, GS], bf16)
                nc.sync.dma_start(whh0T_sb[:], whh0T[:])
                wih1T_sb = P.tile([128, 8, GS], bf16)
                nc.sync.dma_start(wih1T_sb[:], wih1T[:])
                whh1T_sb = P.tile([128, 8, GS], bf16)
                nc.sync.dma_start(whh1T_sb[:], whh1T[:])
                bias1_sb = P.tile([B, GS], bf16)
                nc.sync.dma_start(
                    bias1_sb[:],
                    bass.AP(tensor=bias1.ap().tensor, offset=0,
                            ap=[[0, B], [1, GS]]))
                maskb_sb = P.tile([BL, T], f32)
                nc.sync.dma_start(maskb_sb[:], maskb[:])
                sel_sb = P.tile([B, BL], bf16)
                nc.sync.dma_start(sel_sb[:], sel[:])
                ident_sb = P.tile([128, 128], f32r)
                nc.sync.dma_start(ident_sb[:], ident[:])
                identb_sb = P.tile([B, B], bf16)
                nc.vector.tensor_copy(out=identb_sb[:],
                                      in_=ident_sb[0:B, 0:B].bitcast(f32))
                pre_encT_sb = P.tile([128, 3, BL * T], bf16)

                woutT_sb = P.tile([128, 8, OS], bf16, tag="wout")
                nc.sync.dma_start(woutT_sb[:], woutT[:])
                bout_sb = P.tile([128, OS], f32, tag="bout")
                nc.sync.dma_start(
                    bout_sb[:],
                    bass.AP(tensor=boutsl.ap().tensor, offset=0,
                            ap=[[0, 128], [1, OS]]))
                lab_sb = P.tile([128, nch], f32, tag="lab")
                nc.sync.dma_start(lab_sb[:], labels[:])
                iota_sb = P.tile([128, OS], f32, tag="iota")
                nc.gpsimd.iota(iota_sb[:], pattern=[[1, OS]], base=0,
                               channel_multiplier=0,
                               allow_small_or_imprecise_dtypes=True)
                m_all = P.tile([128, nch], f32, tag="m")
                s_all = P.tile([128, nch], f32, tag="s")
                lg_all = P.tile([128, nch], f32, tag="lg")
                if "log" in _SKIP:
                    nc.vector.memset(m_all[:], 0.0)
                    nc.vector.memset(s_all[:], 1.0)
                    nc.vector.memset(lg_all[:], 0.0)

                c0_sb = P.tile([B, ZS], f32)
                nc.vector.memset(c0_sb[:], 0.0)
                c1_sb = P.tile([B, ZS], f32)
                nc.vector.memset(c1_sb[:], 0.0)

                # column-masked stationary operands for diag e / att_c:
                # dmask[:, ac, sj, :] has only column sj nonzero (= decT),
                # so one PSUM accumulation over sj-groups yields the
                # diagonal [4, T] directly at base partition 0.
                dmask_sb = P.tile([128, 3, BL, BL], bf16, tag="dmask")
                nc.vector.memset(dmask_sb[:], 0.0)
                wmask_sb = P.tile([128, 4, BL, BL], bf16, tag="wmask")
                nc.vector.memset(wmask_sb[:], 0.0)

                # ------------- prologue A: pre_enc -------------
                with (
                    tc.tile_pool(name="prA", bufs=1) as PA_,
                    tc.tile_pool(name="prAps", bufs=1, space="PSUM") as PAP,
                ):
                    hsT_sb = PA_.tile([128, 4, BL * T], bf16, tag="hsT")
                    nc.sync.dma_start(hsT_sb[:], hsT[:])
                    wencT_sb = PA_.tile([128, 4, APAD], bf16, tag="wenc")
                    nc.sync.dma_start(wencT_sb[:], wencT[:])
                    bencp_sb = PA_.tile([128, 3], f32, tag="benc")
                    nc.sync.dma_start(bencp_sb[:], bencp[:])
                    for ac in range(3):
                        ps = PAP.tile([128, BL * T], f32, tag="pe")
                        for dk in range(4):
                            for ns in range(4):
                                nc.tensor.matmul(
                                    ps[:, ns * 512:(ns + 1) * 512],
                                    wencT_sb[:, dk, ac * 128:(ac + 1) * 128],
                                    hsT_sb[:, dk, ns * 512:(ns + 1) * 512],
                                    start=(dk == 0), stop=(dk == 3))
                        nc.scalar.activation(
                            out=pre_encT_sb[:, ac, :], in_=ps[:],
                            func=FT.Tanh, bias=bencp_sb[:, ac:ac + 1],
                            scale=1.0)

                # ------------- prologue B: X0 precompute -------------
                with (
                    tc.tile_pool(name="prB", bufs=2) as PB,
                    tc.tile_pool(name="prB1", bufs=1) as PB1,
                    tc.tile_pool(name="prBps", bufs=2, space="PSUM") as PBP,
                ):
                    wih0pT_sb = PB1.tile([128, 8, GS], bf16, tag="wih0p")
                    nc.sync.dma_start(wih0pT_sb[:], wih0pT[:])
                    x0bias_sb = PB1.tile([128, GS], f32, tag="x0b")
                    nc.sync.dma_start(
                        x0bias_sb[:],
                        bass.AP(tensor=x0bias.ap().tensor, offset=0,
                                ap=[[0, 128], [1, GS]]))
                    x0_flat = x0_dram[:].rearrange("t b g -> (t b) g")
                    for ch in range(nch):
                        cw = min(128, nrow - ch * 128)
                        ey_t = PB.tile([128, 8, 128], bf16, tag="eych")
                        nc.sync.dma_start(
                            ey_t[:, :, :cw],
                            eysT[:, :, ch * 128: ch * 128 + cw])
                        ps = PBP.tile([128, GS], f32, tag="x0")
                        for kt in range(8):
                            nc.tensor.matmul(
                                ps[:cw, :], ey_t[:, kt, :cw],
                                wih0pT_sb[:, kt, :],
                                start=(kt == 0), stop=(kt == 7))
                        g = PB.tile([128, GS], bf16, tag="x0g")
                        nc.vector.tensor_tensor(
                            out=g[:cw, :], in0=ps[:cw, :],
                            in1=x0bias_sb[:cw, :], op=OP.add)
                        nc.sync.dma_start(
                            x0_flat[ch * 128: ch * 128 + cw, :], g[:cw, :])

                # ------------- recurrence + streamed logits -------------
                with (
                    tc.tile_pool(name="work", bufs=2) as W,
                    tc.tile_pool(name="zzp", bufs=3) as ZZ,
                    tc.tile_pool(name="zsl", bufs=2) as ZSL,
                    tc.tile_pool(name="lgb", bufs=2) as LGB,
                    tc.tile_pool(name="ps_big", bufs=2, space="PSUM") as PSbig,
                    tc.tile_pool(name="ps_sm", bufs=2, space="PSUM") as PSsm,
                    tc.tile_pool(name="ps_g0", bufs=2, space="PSUM") as PSg0,
                    tc.tile_pool(name="ps_g1", bufs=2, space="PSUM") as PSg1,
                    tc.tile_pool(name="bnc", bufs=2, space="DRAM") as BN,
                    tc.tile_pool(name="shr", bufs=2, space="DRAM") as SH,
                    tc.tile_pool(name="x0pre", bufs=3) as X0P,
                ):
                    # zz(t) holds gathered [z0T(t) | z1T(t-1)] as
                    # [128, 8, 2B]: cols 0:B = z0, B:2B = z1.
                    zz_prev = ZZ.tile([128, 8, 2 * B], bf16, tag="zz")
                    nc.sync.dma_start(zz_prev[:], zinit[:])
                    zslots_cur = None
                    done_ch = []

                    def logits_chunk(ch, zsrc, cw):
                        buf = LGB.tile([128, OS], f32, tag="lbuf")
                        for (o0, ow) in ((0, 512), (512, 512),
                                         (1024, OS - 1024)):
                            ps = PSbig.tile([128, 512], f32, tag="big")
                            for kt in range(8):
                                nc.tensor.matmul(
                                    ps[:cw, :ow], zsrc[:, kt, :cw],
                                    woutT_sb[:, kt, o0:o0 + ow],
                                    start=(kt == 0), stop=(kt == 7))
                            nc.vector.tensor_tensor(
                                out=buf[:cw, o0:o0 + ow], in0=ps[:cw, :ow],
                                in1=bout_sb[:cw, o0:o0 + ow], op=OP.add)
                        negm = LGB.tile([128, 1], f32, tag="lnegm")
                        nc.vector.tensor_reduce(
                            out=negm[:cw], in_=buf[:cw, :],
                            op=OP.max, axis=AX.X, negate=True)
                        nc.vector.tensor_scalar_mul(
                            out=m_all[:cw, ch:ch + 1], in0=negm[:cw],
                            scalar1=-1.0)
                        mask = LGB.tile([128, OS], f32, tag="lmask")
                        nc.vector.tensor_scalar(
                            out=mask[:cw, :], in0=iota_sb[:cw, :],
                            scalar1=lab_sb[:cw, ch:ch + 1], scalar2=None,
                            op0=OP.is_equal)
                        prod = LGB.tile([128, OS], f32, tag="lprod")
                        nc.vector.tensor_mul(out=prod[:cw, :],
                                             in0=buf[:cw, :],
                                             in1=mask[:cw, :])
                        nc.vector.tensor_reduce(
                            out=lg_all[:cw, ch:ch + 1], in_=prod[:cw, :],
                            op=OP.add, axis=AX.X)
                        nc.scalar.activation(
                            out=buf[:cw, :], in_=buf[:cw, :], func=FT.Exp,
                            bias=negm[:cw], scale=1.0,
                            accum_out=s_all[:cw, ch:ch + 1])
                        done_ch.append(ch)

                    for t in range(steps):
                        x0_t = X0P.tile([B, GS], bf16, tag="x0t")
                        nc.sync.dma_start(x0_t[:], x0_dram[t])

                        # bank z1(t-2) from zz_prev into the chunk slots
                        if t >= 2:
                            sl = (t - 2) % 4
                            if sl == 0:
                                zslots_cur = ZSL.tile([128, 8, 4 * B], bf16,
                                                      tag="zslots")
                            nc.vector.tensor_copy(
                                out=zslots_cur[:, :, sl * B:(sl + 1) * B],
                                in_=zz_prev[:, :, B:2 * B])

                        # ---- attention (uses z0(t-1) = zz_prev cols 0:B)
                        skip_attn = "attn" in _SKIP
                        if not skip_attn:
                         dec_ps = PSsm.tile([B, APAD], f32, tag="sm")
                        for kt in range(8):
                            nc.tensor.matmul(dec_ps[:],
                                             zz_prev[:, kt, 0:B],
                                             wdecT_sb[:, kt, :],
                                             start=(kt == 0), stop=(kt == 7))
                        dec_sb = W.tile([B, APAD], bf16, tag="dec")
                        nc.scalar.activation(out=dec_sb[:], in_=dec_ps[:],
                                             func=FT.Tanh)

                        # decT compact [128, 3, 4] via selector matmul,
                        # written straight onto dmask's diagonal columns
                        dT_ps = PSsm.tile([128, 3, BL], f32, tag="sm")
                        for ac in range(3):
                            nc.tensor.matmul(
                                dT_ps[:, ac, :],
                                dec_sb[:, ac * 128:(ac + 1) * 128],
                                sel_sb[:], start=True, stop=True)
                        dm = dmask_sb[:]
                        nc.vector.tensor_copy(
                            out=bass.AP(tensor=dm.tensor, offset=dm.offset,
                                        ap=[list(dm.ap[0]), [BL * BL, 3],
                                            [BL + 1, BL]]),
                            in_=dT_ps[:])

                        # e diagonal [4, T]: one accumulation over (sj, ac)
                        eP = PSbig.tile([BL, T], f32, tag="big")
                        for sj in range(BL):
                            for ac in range(3):
                                nc.tensor.matmul(
                                    eP[:],
                                    dmask_sb[:, ac, sj, :],
                                    pre_encT_sb[:, ac, sj * T:(sj + 1) * T],
                                    start=(sj == 0 and ac == 0),
                                    stop=(sj == BL - 1 and ac == 2))
                        # w = softmax(2*e + maskb) over T
                        e_b = W.tile([BL, T], f32, tag="eb")
                        nc.vector.scalar_tensor_tensor(
                            out=e_b[:], in0=eP[:], scalar=2.0,
                            in1=maskb_sb[:], op0=OP.mult, op1=OP.add)
                        negm = W.tile([BL, 1], f32, tag="negm")
                        nc.vector.tensor_reduce(
                            out=negm[:], in_=e_b[:], op=OP.max, axis=AX.X,
                            negate=True)
                        w_t = W.tile([BL, T], f32r, tag="wt")
                        ssum = W.tile([BL, 1], f32, tag="ssum")
                        nc.scalar.activation(
                            out=w_t[:], in_=e_b[:], func=FT.Exp,
                            bias=negm[:], scale=1.0, accum_out=ssum[:])
                        rsum = W.tile([BL, 1], f32, tag="rsum")
                        nc.vector.reciprocal(out=rsum[:], in_=ssum[:])

                        # wT (4 transposes [4,128] -> [128,4]) written onto
                        # wmask's diagonal columns
                        wT_ps = PSsm.tile([128, 4, BL], f32r, tag="sm")
                        for tk in range(4):
                            nc.tensor.transpose(
                                wT_ps[:, tk, :],
                                w_t[:, tk * 128:(tk + 1) * 128],
                                ident_sb[0:BL, 0:BL])
                        wm = wmask_sb[:]
                        nc.vector.tensor_copy(
                            out=bass.AP(tensor=wm.tensor, offset=wm.offset,
                                        ap=[list(wm.ap[0]), [BL * BL, 4],
                                            [BL + 1, BL]]),
                            in_=wT_ps[:])

                        # att_c diagonal [4, 512]: one accumulation group
                        aP = PSbig.tile([BL, EPROJS], f32, tag="big")
                        for sj in range(BL):
                            for tk in range(4):
                                nc.tensor.matmul(
                                    aP[:],
                                    wmask_sb[:, tk, sj, :],
                                    hs_sb[:, sj, tk, :],
                                    start=(sj == 0 and tk == 0),
                                    stop=(sj == BL - 1 and tk == 3))
                        # fold the softmax 1/sum into the psum->SBUF copy
                        att_sb = W.tile([BL, EPROJS], f32r, tag="attsb")
                        nc.vector.tensor_scalar_mul(out=att_sb[:], in0=aP[:],
                                                    scalar1=rsum[:])

                        # gather att rows (contiguous): [4,512] -> [32,512]
                        aT_in = BN.tile([BL, EPROJS], f32, tag="aTb")
                        nc.sync.dma_start(aT_in[:], att_sb[:].bitcast(f32))
                        aT_out = SH.tile([B, EPROJS], f32, tag="aTs",
                                         addr_space="Shared")
                        nc.gpsimd.collective_compute(
                            "AllGather", OP.bypass, replica_groups=rg,
                            ins=[aT_in[:]], outs=[aT_out[:]])

                        # ---- fill the collective shadow:
                        # g0 hh-part (z0(t-1)) started now
                        g0_ps = PSg0.tile([B, GS], f32, tag="g0")
                        for kt in range(8):
                            nc.tensor.matmul(g0_ps[:], zz_prev[:, kt, 0:B],
                                             whh0T_sb[:, kt, :],
                                             start=(kt == 0), stop=False)
                        # LSTM1 for step t-1 (z0(t-1), z1(t-2) both local).
                        # At t=0 there is no step -1: send zeros instead.
                        zsl_sb = W.tile([128, 2 * B], bf16, tag="zslb")
                        if t == 0:
                            nc.vector.memset(zsl_sb[:, B:2 * B], 0.0)
                        else:
                            g1_ps = PSg1.tile([B, GS], f32, tag="g1")
                            for kt in range(8):
                                nc.tensor.matmul(
                                    g1_ps[:], zz_prev[:, kt, B:2 * B],
                                    whh1T_sb[:, kt, :],
                                    start=(kt == 0), stop=False)
                            for kt in range(8):
                                nc.tensor.matmul(
                                    g1_ps[:], zz_prev[:, kt, 0:B],
                                    wih1T_sb[:, kt, :],
                                    start=False, stop=False)
                            nc.tensor.matmul(g1_ps[:], identb_sb[:],
                                             bias1_sb[:],
                                             start=False, stop=True)
                            z1n = _cell(nc, W, g1_ps, c1_sb, "c1")
                            z1T_ps = PSsm.tile([128, B], f32r, tag="sm")
                            nc.tensor.transpose(z1T_ps[:], z1n[:],
                                                ident_sb[0:B, 0:B])
                            nc.vector.tensor_copy(out=zsl_sb[:, B:2 * B],
                                                  in_=z1T_ps[:])

                        # ---- consume gathered att rows: [32,512] -> attT
                        attall_sb = W.tile([B, EPROJS], f32, tag="attall")
                        nc.sync.dma_start(attall_sb[:], aT_out[:])
                        aT_ps = PSsm.tile([128, 4, B], f32r, tag="sm")
                        for dk in range(4):
                            nc.tensor.transpose(
                                aT_ps[:, dk, :],
                                attall_sb[:, dk * 128:(dk + 1) * 128]
                                .bitcast(f32r),
                                ident_sb[0:B, 0:B])
                        attT_sb = W.tile([128, 4, B], bf16, tag="attT")
                        nc.vector.tensor_copy(out=attT_sb[:], in_=aT_ps[:])
                        for dk in range(4):
                            nc.tensor.matmul(g0_ps[:], attT_sb[:, dk, :],
                                             wattT_sb[:, dk, :],
                                             start=False, stop=(dk == 3))
                        nc.tensor.matmul(g0_ps[:], identb_sb[:], x0_t[:],
                                         start=False, stop=True)
                        z0n = _cell(nc, W, g0_ps, c0_sb, "c0")
                        z0T_ps = PSsm.tile([128, B], f32r, tag="sm")
                        nc.tensor.transpose(z0T_ps[:], z0n[:],
                                            ident_sb[0:B, 0:B])
                        nc.vector.tensor_copy(out=zsl_sb[:, 0:B],
                                              in_=z0T_ps[:])

                        # ---- merged gather [z0(t) | z1(t-1)]
                        zb_in = BN.tile([128, 2 * B], bf16, tag="zb")
                        nc.sync.dma_start(zb_in[:], zsl_sb[:])
                        # keep the PE HAM-warm through the gather window
                        if t < steps - 1:
                            dmy = PSg1.tile([B, GS], f32, tag="g1")
                            for i in range(WARM):
                                nc.tensor.matmul(
                                    dmy[:], zsl_sb[:, 0:B],
                                    whh0T_sb[:, i % 8, :],
                                    start=True, stop=True)
                        zb_out = SH.tile([128 * NC, 2 * B], bf16, tag="zs",
                                         addr_space="Shared")
                        nc.gpsimd.collective_compute(
                            "AllGather", OP.bypass, replica_groups=rg,
                            ins=[zb_in[:]], outs=[zb_out[:]])
                        zz_new = ZZ.tile([128, 8, 2 * B], bf16, tag="zz")
                        nc.sync.dma_start(
                            zz_new[:],
                            zb_out[:].rearrange("(kt k) b -> k kt b", k=128))
                        zz_prev = zz_new

                        # ---- streamed logits chunk (every 4th step)
                        if (t >= 2 and (t - 2) % 4 == 3 and (t - 2) // 4 < nch
                                and "log" not in _SKIP):
                            logits_chunk((t - 2) // 4, zslots_cur, 128)

                    # ------------- epilogue -------------
                    # z1(steps-1): needs z0(steps-1), z1(steps-2) = zz_prev
                    g1_ps = PSg1.tile([B, GS], f32, tag="g1")
                    for kt in range(8):
                        nc.tensor.matmul(g1_ps[:], zz_prev[:, kt, B:2 * B],
                                         whh1T_sb[:, kt, :],
                                         start=(kt == 0), stop=False)
                    for kt in range(8):
                        nc.tensor.matmul(g1_ps[:], zz_prev[:, kt, 0:B],
                                         wih1T_sb[:, kt, :],
                                         start=False, stop=False)
                    nc.tensor.matmul(g1_ps[:], identb_sb[:], bias1_sb[:],
                                     start=False, stop=True)
                    z1n = _cell(nc, W, g1_ps, c1_sb, "c1")
                    z1T_ps = PSsm.tile([128, B], f32r, tag="sm")
                    nc.tensor.transpose(z1T_ps[:], z1n[:],
                                        ident_sb[0:B, 0:B])
                    z1l_sb = W.tile([128, B], bf16, tag="z1l")
                    nc.vector.tensor_copy(out=z1l_sb[:], in_=z1T_ps[:])
                    zb1_in = BN.tile([128, B], bf16, tag="zb1")
                    nc.sync.dma_start(zb1_in[:], z1l_sb[:])
                    zb1_out = SH.tile([128 * NC, B], bf16, tag="zs1",
                                      addr_space="Shared")
                    nc.gpsimd.collective_compute(
                        "AllGather", OP.bypass, replica_groups=rg,
                        ins=[zb1_in[:]], outs=[zb1_out[:]])
                    zlast = ZZ.tile([128, 8, B], bf16, tag="zlast")
                    nc.sync.dma_start(
                        zlast[:],
                        zb1_out[:].rearrange("(kt k) b -> k kt b", k=128))

                    # remaining chunks.  In-loop banking covered z1 indices
                    # 0..steps-3; z1(steps-2) is in zz_prev cols B:2B and
                    # z1(steps-1) is zlast.
                    last_banked_chunk = (steps - 3) // 4 if steps >= 3 else -1
                    for ch in range(nch):
                        if ch in done_ch or "log" in _SKIP:
                            continue
                        cw = min(128, nrow - ch * 128)
                        if ch == last_banked_chunk and zslots_cur is not None:
                            zt = zslots_cur
                        else:
                            zt = ZSL.tile([128, 8, 4 * B], bf16,
                                          tag="zslots")
                        for j in range(4):
                            tt = 4 * ch + j          # step index
                            if tt >= steps:
                                break
                            if tt == steps - 2:
                                nc.vector.tensor_copy(
                                    out=zt[:, :, j * B:(j + 1) * B],
                                    in_=zz_prev[:, :, B:2 * B])
                            elif tt == steps - 1:
                                nc.vector.tensor_copy(
                                    out=zt[:, :, j * B:(j + 1) * B],
                                    in_=zlast[:])
                            # tt <= steps-3: already banked in zt in-loop
                        logits_chunk(ch, zt, cw)

                    nc.sync.dma_start(out_stats[:, :, 0], m_all[:])
                    nc.sync.dma_start(out_stats[:, :, 1], s_all[:])
                    nc.sync.dma_start(out_stats[:, :, 2], lg_all[:])

    nc.finalize()
    return nc


# ---------------------------------------------------------------------------
# host side
# ---------------------------------------------------------------------------

def _prep_inputs(hs_pad, hlens, ys_pad, embed_w, Wenc, benc, Wdec,
                 W_ih0, W_hh0, b_ih0, b_hh0, W_ih1, W_hh1, b_ih1, b_hh1,
                 Wout, bout, steps):
    """Shard + pack all inputs into per-core in_maps (pure data movement).

    z-states are stored doubled on device (Z = 2z), so every weight that
    multiplies a hidden state (Wdec, Whh0, Wih1, Whh1, Wout) is halved.
    """
    f = np.float32
    bf = ml_dtypes.bfloat16
    hs_pad = np.asarray(hs_pad, f)
    ys_pad = np.asarray(ys_pad)
    ys_in = np.concatenate(
        [np.full((B, 1), SOS, ys_pad.dtype), ys_pad], axis=1)[:, :steps]
    ys_out = np.concatenate(
        [ys_pad, np.full((B, 1), EOS, ys_pad.dtype)], axis=1)[:, :steps]

    # gate permutation: core c's rows = 128 each of i/f/g/o
    perm = np.concatenate(
        [g * DUNITS + c * ZS + np.arange(ZS)
         for c in range(NC) for g in range(4)])

    eys = np.asarray(embed_w, f)[ys_in]                  # [B, steps, 1024]
    eysT = np.ascontiguousarray(
        eys.transpose(2, 1, 0).reshape(DUNITS, steps * B))
    eysT = np.ascontiguousarray(
        eysT.reshape(8, 128, -1).transpose(1, 0, 2)).astype(bf)

    def kpack(M, dt=bf):
        """[K, N] -> [128, K//128, N]"""
        K = M.shape[0]
        return np.ascontiguousarray(
            M.reshape(K // 128, 128, -1).transpose(1, 0, 2)).astype(dt)

    W_ih0 = np.asarray(W_ih0, f)[perm]
    W_hh0 = np.asarray(W_hh0, f)[perm] * 0.5
    W_ih1 = np.asarray(W_ih1, f)[perm] * 0.5
    W_hh1 = np.asarray(W_hh1, f)[perm] * 0.5
    bias0 = (np.asarray(b_ih0, f) + np.asarray(b_hh0, f))[perm]
    bias1v = (np.asarray(b_ih1, f) + np.asarray(b_hh1, f))[perm]

    wencp = np.zeros((APAD, EPROJS), f)
    wencp[:ATT_DIM] = np.asarray(Wenc, f)
    bencpv = np.zeros((3, 128), f)
    bencpv.reshape(-1)[:ATT_DIM] = np.asarray(benc, f)
    wdecp = np.zeros((APAD, DUNITS), f)
    wdecp[:ATT_DIM] = np.asarray(Wdec, f) * 0.5

    wencT = kpack(wencp.T)                      # [128, 4, 384]
    wdecT = kpack(wdecp.T)                      # [128, 8, 384]
    identv = np.eye(128, dtype=f)
    zinitv = np.zeros((128, 8, 2 * B), bf)

    Wout = np.asarray(Wout, f) * 0.5
    bout_v = np.asarray(bout, f)

    ys_out_flat = ys_out.T.reshape(-1)          # row r = t*B + b
    nrow = steps * B
    nch = (nrow + 127) // 128

    in_maps = []
    for c in range(NC):
        sl = slice(GS * c, GS * (c + 1))
        seqs = slice(BL * c, BL * (c + 1))
        hs_c = hs_pad[seqs]                     # [4, 512, 512]
        hs_nat = np.ascontiguousarray(
            hs_c.reshape(BL, 4, 128, EPROJS).transpose(2, 0, 1, 3)).astype(bf)
        hsT = np.ascontiguousarray(
            hs_c.transpose(2, 0, 1)             # [d, s, t]
            .reshape(4, 128, BL, T)
            .transpose(1, 0, 2, 3)
            .reshape(128, 4, BL * T)).astype(bf)
        hl = np.asarray(hlens).reshape(-1)[seqs]
        maskbv = np.where(np.arange(T)[None, :] < hl[:, None],
                          0.0, -1e10).astype(f)
        selv = np.zeros((B, BL), f)
        for j in range(BL):
            selv[BL * c + j, j] = 1.0
        labv = np.full((nch * 128,), -1.0, f)
        lo = OS * c
        lb = ys_out_flat.astype(np.int64) - lo
        valid = (lb >= 0) & (lb < OS)
        labv[:nrow][valid] = lb[valid].astype(f)
        labv = labv.reshape(nch, 128).T.copy()  # [128, nch]

        in_maps.append({
            "hs_nat": hs_nat,
            "hsT": hsT,
            "eysT": eysT,
            "wih0pT": kpack(W_ih0[sl, :DUNITS].T),
            "x0bias": np.ascontiguousarray(bias0[sl][None]),
            "wencT": wencT,
            "bencp": np.ascontiguousarray(bencpv.T),
            "wdecT": wdecT,
            "wattT": kpack(W_ih0[sl, DUNITS:].T),
            "whh0T": kpack(W_hh0[sl].T),
            "wih1T": kpack(W_ih1[sl].T),
            "whh1T": kpack(W_hh1[sl].T),
            "bias1": np.ascontiguousarray(bias1v[sl][None]).astype(bf),
            "maskb": maskbv,
            "sel": selv.astype(bf),
            "woutT": kpack(Wout[OS * c: OS * (c + 1)].T),
            "boutsl": np.ascontiguousarray(bout_v[OS * c: OS * (c + 1)][None]),
            "labels": labv,
            "ident": identv,
            "zinit": zinitv,
        })
    return in_maps


def _combine(results, steps):
    """Merge per-core (m, S, lab) partials into (loss, acc, ppl)."""
    nrow = steps * B
    ms, ss, labs = [], [], []
    for r in results:
        st = r["out_stats"]                     # [128, nch, 3]
        ms.append(st[:, :, 0].T.reshape(-1)[:nrow])
        ss.append(st[:, :, 1].T.reshape(-1)[:nrow])
        labs.append(st[:, :, 2].T.reshape(-1)[:nrow])
    m = np.stack(ms)
    s = np.stack(ss)
    lab = np.stack(labs)
    gmax = m.max(axis=0)
    gsum = (s.astype(np.float64)
            * np.exp(m.astype(np.float64) - gmax[None])).sum(axis=0)
    lablogit = lab.sum(axis=0)
    nll = gmax.astype(np.float64) + np.log(gsum) - lablogit
    match = (lab == gmax[None]).any(axis=0)
    loss = np.float32(nll.mean() * L)
    acc = np.float32(match.mean())
    ppl = np.float32(np.exp(np.float64(loss) / B))
    return loss, acc, ppl


def kernel(**inputs):
    steps = S
    in_maps = _prep_inputs(steps=steps, **inputs)
    if steps not in _BUILD_CACHE:
        _BUILD_CACHE[steps] = build(steps)
    nc = _BUILD_CACHE[steps]
    res = bass_utils.run_bass_kernel_spmd(
        nc, in_maps, core_ids=list(range(NC)))
    return _combine(res.results, steps)
